# revision 19
# baseline (speedup 1.0000x reference)
"""Trainium2 Bass kernel for the snake-ordered lattice GRU wavefunction model.

v2 strategy (data-parallel over batch, 8 cores x 128 samples):
  - 64 sequential sites; per site pre = st @ W_sel decomposed as
    base + sum_i s_i * (st @ D_i), with the bias folded into the base
    PSUM group via a ones-row k-tile (no vector bias add).
  - Contraction split by neighbor half:
      * y-half (hy, available ~15 sites early): fp8-e4m3 DoubleRow matmuls
        (ring_fp8 = 16*h, gate weights 64*D) -> 4x fewer PE cycles.
      * x-half (hx, on the critical chain): bf16 matmuls with gate weights
        pre-scaled 1024x so both halves accumulate consistently in PSUM;
        the gating scalars are divided by 1024 on the host.
  - Gating MACs read gate PSUM banks directly and are split across the
    Vector and Pool engines (2 gates each) into two partial accumulators,
    then joined quarter-wise so tanh/sigmoid can start ASAP.
  - tanh/sig/du/transpose tail runs at 128-column granularity per k-slice
    so ring k0 lands (and the next site's x GEMMs start) before the k1
    sub-chain finishes.
  - h^T is produced by PSUM-accumulated transposes T(ms)+T(u*(h~-ms));
    ring copies cast to bf16 (Vector: k0, Scalar: k1) and to fp8 (Scalar).
  - Head (h @ [Wl1|Wl2]) folded into chunk7 of the NEXT site's GEMM.
  - Softmax/sector-mask/log accumulation runs on host (O(B*64*3)).
"""
import os
import sys
import numpy as np

sys.path.insert(0, '/opt/trn_rl_repo')

B, NX, NY, I, H = 1024, 8, 8, 3, 256
N_TARGET, SZ = 48, 0
NCORES = 8
BC = B // NCORES          # 128 samples per core
NSITES = NX * NY          # 64
RING = 16                 # h ring buffer depth (max hy lookback is 15)

GSCALE = 4096.0           # x-gate weight prescale (folded out via sxy/GSCALE)
Y8_WSCALE = 64.0          # fp8 y-gate weight prescale
Y8_HSCALE = 64.0          # fp8 ring scale
USE_FP8_Y = os.environ.get("BASS_FP8Y", "0") == "1"

_cached = {}


def _snake_sites():
    sites = []
    for ny in range(NY):
        xs = range(NX) if ny % 2 == 0 else range(NX - 1, -1, -1)
        dx = -1 if ny % 2 == 0 else 1
        for nx in xs:
            sites.append((nx, ny, nx + dx))
    return sites


SITES = _snake_sites()


def _build_program():
    import concourse.tile as tile
    from concourse import bacc, mybir

    f32 = mybir.dt.float32
    bf16 = mybir.dt.bfloat16
    fp8 = mybir.dt.float8e4
    Alu = mybir.AluOpType
    Act = mybir.ActivationFunctionType
    DR = mybir.MatmulPerfMode.DoubleRow

    nc = bacc.Bacc("TRN2", target_bir_lowering=False, debug=False,
                   num_devices=NCORES)

    wgx_d = nc.dram_tensor("wgx", [256, 2048], f32, kind="ExternalInput").ap()
    wgyb_d = nc.dram_tensor("wgyb", [256, 2048], f32, kind="ExternalInput").ap()
    wgy8_d = nc.dram_tensor("wgy8", [128, 4096], f32, kind="ExternalInput").ap()
    wbf_d = nc.dram_tensor("wbf", [512, 512], f32, kind="ExternalInput").ap()
    wbx_d = nc.dram_tensor("wbx", [256, 512], f32, kind="ExternalInput").ap()
    wby_d = nc.dram_tensor("wby", [256, 512], f32, kind="ExternalInput").ap()
    wc7a_d = nc.dram_tensor("wc7a", [512, 262], f32, kind="ExternalInput").ap()
    wc7b_d = nc.dram_tensor("wc7b", [512, 262], f32, kind="ExternalInput").ap()
    brow_d = nc.dram_tensor("brow", [128, 512], f32, kind="ExternalInput").ap()
    ones0_d = nc.dram_tensor("ones0", [128, 128], f32, kind="ExternalInput").ap()
    ident_d = nc.dram_tensor("ident", [128, 128], f32, kind="ExternalInput").ap()
    sxy_d = nc.dram_tensor("sxy", [128, NSITES * 4], f32, kind="ExternalInput").ap()
    logits_d = nc.dram_tensor("logits", [128, NSITES * 6], f32,
                              kind="ExternalOutput").ap()

    with tile.TileContext(nc) as tc:
        with (
            tc.tile_pool(name="const", bufs=1) as constp,
            tc.tile_pool(name="work", bufs=4) as workp,
            tc.tile_pool(name="psc", bufs=1, space="PSUM") as pscp,
        ):
            # ---- persistent SBUF tiles ----
            wgx_sb = [constp.tile([128, 2048], bf16, tag=f"wgx{k}", name=f"wgx{k}") for k in range(2)]
            wgyb_sb = [constp.tile([128, 2048], bf16, tag=f"wgyb{k}", name=f"wgyb{k}") for k in range(2)]
            wgy8_sb = [constp.tile([128, 1024], fp8, tag=f"wgy8{c}", name=f"wgy8{c}") for c in range(4)]
            wbf_sb = [constp.tile([128, 512], bf16, tag=f"wbf{k}", name=f"wbf{k}") for k in range(4)]
            wbx_sb = [constp.tile([128, 512], bf16, tag=f"wbx{k}", name=f"wbx{k}") for k in range(2)]
            wby_sb = [constp.tile([128, 512], bf16, tag=f"wby{k}", name=f"wby{k}") for k in range(2)]
            wc7a_sb = [constp.tile([128, 262], bf16, tag=f"wa{k}", name=f"wa{k}") for k in range(4)]
            wc7b_sb = [constp.tile([128, 262], bf16, tag=f"wb{k}", name=f"wb{k}") for k in range(4)]
            brow_sb = constp.tile([128, 512], bf16, tag="brow")
            ones0_sb = constp.tile([128, 128], bf16, tag="ones0")
            ident_sb = constp.tile([128, 128], f32, tag="ident")
            sxy_sb = constp.tile([128, NSITES * 4], f32, tag="sxy")
            ring_sb = constp.tile([128, RING * 256], bf16, tag="ring")
            ring8_sb = constp.tile([128, RING * 256], fp8, tag="ring8")
            logit_sb = constp.tile([128, NSITES * 6], f32, tag="lstage")

            # ---- persistent PSUM tiles (8 banks exactly) ----
            cb_ps = pscp.tile([128, 512], f32, tag="cb")
            g_ps = [pscp.tile([128, 512], f32, tag=f"g{c}", name=f"gps{c}")
                    for c in range(4)]
            c7_ps = [pscp.tile([128, 262], f32, tag=f"c7{i}", name=f"c7ps{i}")
                     for i in range(2)]
            tr_ps = pscp.tile([128, 256], f32, tag="tr")

            # ---- loads (f32 staging -> cast) ----
            def load_cast(dst, src_ap, k, dt):
                stg = workp.tile(list(dst.shape), f32, tag="wstage",
                                 name=f"wstage{k}", bufs=3)
                nc.sync.dma_start(stg[:], src_ap)
                nc.vector.tensor_copy(dst[:], stg[:])

            li = 0
            for k in range(2):
                rows = slice(128 * k, 128 * (k + 1))
                load_cast(wgx_sb[k], wgx_d[rows, :], li, bf16); li += 1
                load_cast(wbx_sb[k], wbx_d[rows, :], li, bf16); li += 1
                load_cast(wc7a_sb[k], wc7a_d[rows, :], li, bf16); li += 1
                load_cast(wbf_sb[k], wbf_d[rows, :], li, bf16); li += 1
            for k in range(2):
                rows = slice(128 * k, 128 * (k + 1))
                rows2 = slice(128 * (k + 2), 128 * (k + 3))
                load_cast(wgyb_sb[k], wgyb_d[rows, :], li, bf16); li += 1
                load_cast(wby_sb[k], wby_d[rows, :], li, bf16); li += 1
                load_cast(wbf_sb[k + 2], wbf_d[rows2, :], li, bf16); li += 1
                load_cast(wc7a_sb[k + 2], wc7a_d[rows2, :], li, bf16); li += 1
            for k in range(4):
                rows = slice(128 * k, 128 * (k + 1))
                load_cast(wc7b_sb[k], wc7b_d[rows, :], li, bf16); li += 1
            for c in range(4):
                load_cast(wgy8_sb[c], wgy8_d[:, 1024 * c:1024 * (c + 1)], li, fp8)
                li += 1
            load_cast(brow_sb, brow_d[:], li, bf16); li += 1
            load_cast(ones0_sb, ones0_d[:], li, bf16); li += 1
            nc.sync.dma_start(ident_sb[:], ident_d[:])
            nc.sync.dma_start(sxy_sb[:], sxy_d[:])

            def ring_k(site, k):
                base = (site % RING) * 256 + 128 * k
                return ring_sb[:, base:base + 128]

            def ring8_pair(site):
                base = (site % RING) * 256
                return ring8_sb[:, base:base + 256].rearrange(
                    "p (j b) -> p j b", j=2)

            def mm(out, lhsT, rhs, start, stop, pm=None):
                nc.tensor.matmul(out, lhsT, rhs, start=start, stop=stop,
                                 perf_mode=pm)

            # per-site state handed to the next iteration
            du_t = {}   # (du0, du1) tiles
            ms_t = {}

            for t, (nx, ny, nxn) in enumerate(SITES):
                x_act = (t % 8 != 0)
                y_act = (t >= 8)
                interior = x_act and y_act
                ta = 8 * ny - 1 - (t % 8) if y_act else -1
                w7 = wc7a_sb if (x_act or t == 0) else wc7b_sb

                def finish_prev():
                    """T(du0,t-1); T(ms1,t-1); T(du1,t-1) + ring copies.

                    Group order keeps at most one pending accumulation group
                    in the tr bank's zero region at any time.
                    """
                    pdu0, pdu1 = du_t[t - 1]
                    st0 = (t - 1 == 0)  # site0 had no ms -> start here
                    nc.tensor.matmul(tr_ps[:, 0:128], pdu0[:], ident_sb[:],
                                     is_transpose=True, start=st0, stop=True)
                    nc.vector.tensor_copy(ring_k(t - 1, 0), tr_ps[:, 0:128])
                    if not st0:
                        pms = ms_t[t - 1]
                        nc.tensor.matmul(tr_ps[:, 128:256], pms[:, 128:256],
                                         ident_sb[:], is_transpose=True,
                                         start=True, stop=False)
                    nc.tensor.matmul(tr_ps[:, 128:256], pdu1[:], ident_sb[:],
                                     is_transpose=True, start=st0, stop=True)
                    nc.vector.tensor_copy(ring_k(t - 1, 1), tr_ps[:, 128:256])
                    if USE_FP8_Y:
                        nc.scalar.activation(
                            ring8_sb[:, ((t - 1) % RING) * 256:
                                     ((t - 1) % RING) * 256 + 256],
                            tr_ps[:, 0:256], Act.Copy, scale=Y8_HSCALE)

                # ---- matmul emission ----
                if t == 0:
                    # pre = bias only; no gates, no c7
                    mm(cb_ps[:], ones0_sb[:], brow_sb[:], True, True)
                elif not y_act:
                    # x-only: contract hx (k0,k1) only
                    finish_prev()
                    xk = [ring_k(t - 1, 0), ring_k(t - 1, 1)]
                    mm(cb_ps[:], ones0_sb[:], brow_sb[:], True, False)
                    for c in (0, 1):
                        mm(g_ps[c][:], xk[0], wgx_sb[0][:, 512 * c:512 * (c + 1)],
                           True, False)
                        mm(g_ps[c][:], xk[1], wgx_sb[1][:, 512 * c:512 * (c + 1)],
                           False, True)
                    mm(cb_ps[:], xk[0], wbx_sb[0][:], False, False)
                    mm(cb_ps[:], xk[1], wbx_sb[1][:], False, True)
                    mm(c7_ps[t % 2][:], xk[0], w7[0][:], True, False)
                    mm(c7_ps[t % 2][:], xk[1], w7[1][:], False, True)
                elif not x_act:
                    # y-only: contract hy (= h(t-1)) via bf16 k0,k1 of ring
                    finish_prev()
                    yk = [ring_k(t - 1, 0), ring_k(t - 1, 1)]
                    mm(cb_ps[:], ones0_sb[:], brow_sb[:], True, False)
                    for c in (2, 3):
                        mm(g_ps[c][:], yk[0],
                           wgyb_sb[0][:, 512 * c:512 * (c + 1)], True, False)
                        mm(g_ps[c][:], yk[1],
                           wgyb_sb[1][:, 512 * c:512 * (c + 1)], False, True)
                    mm(cb_ps[:], yk[0], wby_sb[0][:], False, False)
                    mm(cb_ps[:], yk[1], wby_sb[1][:], False, True)
                    mm(c7_ps[t % 2][:], yk[0], w7[2][:], True, False)
                    mm(c7_ps[t % 2][:], yk[1], w7[3][:], False, True)
                else:
                    # interior: y-window first (old ring data, frees as the
                    # previous site's MACs drain the banks), then T(du,t-1),
                    # then x-window k0-series / k1-series
                    yk = [ring_k(ta, 0), ring_k(ta, 1)]
                    xk = [ring_k(t - 1, 0), ring_k(t - 1, 1)]
                    mm(c7_ps[t % 2][:], yk[0], w7[2][:], True, False)
                    mm(c7_ps[t % 2][:], yk[1], w7[3][:], False, False)
                    mm(cb_ps[:], ones0_sb[:], brow_sb[:], True, False)
                    mm(cb_ps[:], yk[0], wbf_sb[2][:], False, False)
                    mm(cb_ps[:], yk[1], wbf_sb[3][:], False, False)
                    if USE_FP8_Y:
                        r8 = ring8_pair(ta)
                        for c in (0, 2, 1, 3):
                            rhs8 = wgy8_sb[c][:].rearrange("p (j w) -> p j w", j=2)
                            mm(g_ps[c][:], r8, rhs8, True, False, pm=DR)
                    else:
                        for c in (0, 2, 1, 3):
                            mm(g_ps[c][:], yk[0],
                               wgyb_sb[0][:, 512 * c:512 * (c + 1)], True, False)
                            mm(g_ps[c][:], yk[1],
                               wgyb_sb[1][:, 512 * c:512 * (c + 1)], False, False)
                    finish_prev()
                    # x k0-series: gates first (the MAC chain consumes them
                    # in order g0,g1,g2,cb; g3 drained on Scalar; c7 last)
                    for c in (0, 1, 2):
                        mm(g_ps[c][:], xk[0],
                           wgx_sb[0][:, 512 * c:512 * (c + 1)], False, False)
                    mm(cb_ps[:], xk[0], wbf_sb[0][:], False, False)
                    mm(g_ps[3][:], xk[0], wgx_sb[0][:, 1536:2048], False, False)
                    mm(c7_ps[t % 2][:], xk[0], w7[0][:], False, False)
                    # x k1-series
                    for c in (0, 1, 2):
                        mm(g_ps[c][:], xk[1],
                           wgx_sb[1][:, 512 * c:512 * (c + 1)], False, True)
                    mm(cb_ps[:], xk[1], wbf_sb[1][:], False, True)
                    mm(g_ps[3][:], xk[1], wgx_sb[1][:, 1536:2048], False, True)
                    mm(c7_ps[t % 2][:], xk[1], w7[1][:], False, True)

                # ---- gating MACs (PSUM readable only by Vector + Scalar) ----
                def sc(i):
                    return sxy_sb[:, 4 * t + i:4 * t + i + 1]

                tv = workp.tile([128, 512], f32, tag="tv")
                quarters = []
                if t == 0:
                    quarters = [cb_ps[:, 0:128], cb_ps[:, 128:256],
                                cb_ps[:, 256:384], cb_ps[:, 384:512]]
                elif not interior:
                    ga, gb = (0, 1) if x_act else (2, 3)
                    nc.vector.tensor_scalar_mul(tv[:], g_ps[ga][:], sc(ga))
                    nc.vector.scalar_tensor_tensor(
                        tv[:], g_ps[gb][:], sc(gb), tv[:], Alu.mult, Alu.add)
                    nc.vector.scalar_tensor_tensor(
                        tv[:], cb_ps[:], 1.0, tv[:], Alu.mult, Alu.add)
                    quarters = [tv[:, 0:128], tv[:, 128:256],
                                tv[:, 256:384], tv[:, 384:512]]
                else:
                    gs3 = workp.tile([128, 512], f32, tag="gs3")
                    nc.vector.tensor_scalar_mul(tv[:], g_ps[0][:], sc(0))
                    nc.vector.scalar_tensor_tensor(
                        tv[:], g_ps[1][:], sc(1), tv[:], Alu.mult, Alu.add)
                    nc.vector.scalar_tensor_tensor(
                        tv[:], g_ps[2][:], sc(2), tv[:], Alu.mult, Alu.add)
                    nc.vector.scalar_tensor_tensor(
                        tv[:], cb_ps[:], 1.0, tv[:], Alu.mult, Alu.add)
                    nc.scalar.activation(gs3[:], g_ps[3][:], Act.Identity,
                                         bias=0.0, scale=sc(3))
                    a0 = workp.tile([128, 128], f32, tag="a0")
                    a1 = workp.tile([128, 128], f32, tag="a1")
                    a2 = workp.tile([128, 128], f32, tag="a2")
                    a3 = workp.tile([128, 128], f32, tag="a3")
                    nc.vector.tensor_tensor(a0[:], tv[:, 0:128], gs3[:, 0:128],
                                            Alu.add)
                    nc.vector.tensor_tensor(a2[:], tv[:, 256:384],
                                            gs3[:, 256:384], Alu.add)
                    nc.gpsimd.tensor_tensor(a1[:], tv[:, 128:256],
                                            gs3[:, 128:256], Alu.add)
                    nc.gpsimd.tensor_tensor(a3[:], tv[:, 384:512],
                                            gs3[:, 384:512], Alu.add)
                    quarters = [a0[:], a1[:], a2[:], a3[:]]

                # ---- ms + logit copies (Scalar) ----
                if t >= 1:
                    ms = workp.tile([128, 256], f32, tag="ms")
                    nc.scalar.copy(ms[:], c7_ps[t % 2][:, 0:256])
                    nc.scalar.copy(logit_sb[:, 6 * (t - 1):6 * t],
                                   c7_ps[t % 2][:, 256:262])
                    ms_t[t] = ms

                # ---- activations + du (quarter-pipelined) ----
                ht0 = workp.tile([128, 128], f32, tag="ht0")
                ht1 = workp.tile([128, 128], f32, tag="ht1")
                u0 = workp.tile([128, 128], f32, tag="u0")
                u1 = workp.tile([128, 128], f32, tag="u1")
                du0 = workp.tile([128, 128], f32, tag="du0")
                du1 = workp.tile([128, 128], f32, tag="du1")
                nc.scalar.activation(ht0[:], quarters[0], Act.Tanh)
                nc.scalar.activation(u0[:], quarters[2], Act.Sigmoid)
                if t == 0:
                    nc.vector.tensor_tensor(du0[:], ht0[:], u0[:], Alu.mult)
                else:
                    ms = ms_t[t]
                    nc.vector.tensor_tensor(du0[:], ht0[:], ms[:, 0:128],
                                            Alu.subtract)
                    nc.vector.tensor_tensor(du0[:], du0[:], u0[:], Alu.mult)
                nc.scalar.activation(ht1[:], quarters[1], Act.Tanh)
                nc.scalar.activation(u1[:], quarters[3], Act.Sigmoid)
                if t == 0:
                    nc.gpsimd.tensor_tensor(du1[:], ht1[:], u1[:], Alu.mult)
                else:
                    ms = ms_t[t]
                    nc.gpsimd.tensor_tensor(du1[:], ht1[:], ms[:, 128:256],
                                            Alu.subtract)
                    nc.gpsimd.tensor_tensor(du1[:], du1[:], u1[:], Alu.mult)
                du_t[t] = (du0, du1)

                # ---- T(ms k0) region start; k1 deferred to next finish_prev
                if t >= 1:
                    ms = ms_t[t]
                    nc.tensor.matmul(tr_ps[:, 0:128], ms[:, 0:128], ident_sb[:],
                                     is_transpose=True, start=True, stop=False)

            # ---- tail: transposes + ring + head for site 63 ----
            tl = NSITES - 1
            pdu0, pdu1 = du_t[tl]
            nc.tensor.matmul(tr_ps[:, 0:128], pdu0[:], ident_sb[:],
                             is_transpose=True, start=False, stop=True)
            nc.tensor.matmul(tr_ps[:, 128:256], ms_t[tl][:, 128:256],
                             ident_sb[:], is_transpose=True,
                             start=True, stop=False)
            nc.tensor.matmul(tr_ps[:, 128:256], pdu1[:], ident_sb[:],
                             is_transpose=True, start=False, stop=True)
            nc.vector.tensor_copy(ring_k(tl, 0), tr_ps[:, 0:128])
            nc.vector.tensor_copy(ring_k(tl, 1), tr_ps[:, 128:256])
            nc.tensor.matmul(c7_ps[0][:], ring_k(tl, 0), wc7a_sb[0][:],
                             start=True, stop=False)
            nc.tensor.matmul(c7_ps[0][:], ring_k(tl, 1), wc7a_sb[1][:],
                             start=False, stop=True)
            nc.scalar.copy(logit_sb[:, 6 * tl:6 * (tl + 1)],
                           c7_ps[0][:, 256:262])

            nc.sync.dma_start(logits_d[:], logit_sb[:])

    nc.compile()
    return nc


def _host_pre(samples, W1, W2, Wmerge, Wl1, Wl2, b1, b2):
    oh = np.zeros((B, NX, NY, I), np.float32)
    idx = np.indices(samples.shape)
    oh[idx[0], idx[1], idx[2], samples] = 1.0
    SX = np.zeros((NSITES, B, I), np.float32)
    SY = np.zeros((NSITES, B, I), np.float32)
    for t, (nx, ny, nxn) in enumerate(SITES):
        if 0 <= nxn < NX:
            SX[t] = oh[:, nxn, ny]
        if ny > 0:
            SY[t] = oh[:, nx, ny - 1]

    def DD(i, b):
        return np.concatenate([W1[i] - W1[b], W2[i] - W2[b]], axis=1)

    wg = np.concatenate([DD(1, 0), DD(2, 0), DD(4, 3), DD(5, 3)], axis=1)
    wgx = GSCALE * wg[0:256]                      # [256, 2048]
    wgyb = GSCALE * wg[256:512]                   # [256, 2048]
    # fp8 DR pack: wgy8[p, c*1024 + j*512 + o] = 64*wg[256+j*128+p, 512c+o]
    wgy8 = np.empty((128, 4096), np.float32)
    for c in range(4):
        for j in range(2):
            wgy8[:, c * 1024 + j * 512:c * 1024 + (j + 1) * 512] = \
                Y8_WSCALE * wg[256 + j * 128:256 + (j + 1) * 128,
                               512 * c:512 * (c + 1)]
    wbf = np.concatenate([W1[0] + W1[3], W2[0] + W2[3]], axis=1)
    wbx = np.concatenate([W1[0], W2[0]], axis=1)[0:256]
    wby = np.concatenate([W1[3], W2[3]], axis=1)[256:512]
    Wl = np.concatenate([Wl1, Wl2], axis=1)
    z = np.zeros((H, 6), np.float32)
    wc7a = np.concatenate([Wmerge, np.concatenate([Wl, z], 0)], axis=1)
    wc7b = np.concatenate([Wmerge, np.concatenate([z, Wl], 0)], axis=1)
    brow = np.zeros((128, 512), np.float32)
    brow[0] = np.concatenate([b1, b2])
    ones0 = np.zeros((128, 128), np.float32)
    ones0[0] = 1.0
    c = np.ascontiguousarray
    return (SX, SY, c(wgx), c(wgyb), c(wgy8), c(wbf), c(wbx), c(wby),
            c(wc7a), c(wc7b), brow, ones0)


def _host_post(samples, logits, bl1, bl2):
    """logits: [B, NSITES, 6].  Returns (0.5*log_a, log_p)."""
    log_a = np.zeros(B, np.float32)
    log_p = np.zeros(B, np.float32)
    bl_up = (N_TARGET + 2 * SZ) // 2
    bl_dn = (N_TARGET - 2 * SZ) // 2
    bl_hole = NX * NY - N_TARGET
    n_up = np.zeros(B, np.float32)
    n_dn = np.zeros(B, np.float32)
    ar = np.arange(B)
    for t, (nx, ny, nxn) in enumerate(SITES):
        l1 = logits[:, t, 0:3] + bl1
        l2 = logits[:, t, 3:6] + bl2
        e = np.exp(l1 - l1.max(axis=1, keepdims=True))
        probs = e / e.sum(axis=1, keepdims=True)
        phase = np.float32(np.pi) * (l2 / (1.0 + np.abs(l2)))
        m_up = (bl_up - n_up > 0).astype(np.float32)
        m_dn = (bl_dn - n_dn > 0).astype(np.float32)
        m_hole = (bl_hole - (t - n_up - n_dn) > 0).astype(np.float32)
        mask = np.stack([m_hole, m_dn, m_up], axis=1)
        amp = probs * mask
        amp = amp / np.maximum(amp.sum(axis=1, keepdims=True), 1e-30)
        s = samples[:, nx, ny]
        log_a += np.log(np.clip(amp[ar, s], 1e-12, None)).astype(np.float32)
        log_p += phase[ar, s].astype(np.float32)
        n_up += (s == 2)
        n_dn += (s == 1)
    return (0.5 * log_a).astype(np.float32), log_p.astype(np.float32)


last_results = None  # exposed for test.py profiling


def _install_neff_saver(dst_dir):
    """Monkeypatch bass2jax's BIR->NEFF compile to retain a NEFF copy for
    neuron-profile (the axon path normally discards it)."""
    import shutil
    from concourse import bass2jax as b2j
    if getattr(b2j, "_neff_saver_installed", False):
        return
    orig = b2j.compile_bir_kernel

    def wrapper(bir_json, tmpdir, neff_name="file.neff", **kw):
        out = orig(bir_json, tmpdir, neff_name=neff_name, **kw)
        try:
            shutil.copy(out, os.path.join(dst_dir, "kernel.neff"))
        except Exception:
            pass
        return out

    b2j.compile_bir_kernel = wrapper
    b2j._neff_saver_installed = True


def kernel(samples, W1, b1, W2, b2, Wmerge, Wl1, bl1, Wl2, bl2):
    global last_results
    from concourse.bass_utils import run_bass_kernel_spmd

    samples = np.asarray(samples).astype(np.int64)
    f = lambda x: np.asarray(x, dtype=np.float32)
    W1, b1, W2, b2 = f(W1), f(b1), f(W2), f(b2)
    Wmerge, Wl1, bl1, Wl2, bl2 = f(Wmerge), f(Wl1), f(bl1), f(Wl2), f(bl2)

    (SX, SY, wgx, wgyb, wgy8, wbf, wbx, wby, wc7a, wc7b, brow,
     ones0) = _host_pre(samples, W1, W2, Wmerge, Wl1, Wl2, b1, b2)

    if "nc" not in _cached:
        _cached["nc"] = _build_program()
    nc = _cached["nc"]

    ident = np.eye(128, dtype=np.float32)
    core_ids = list(range(NCORES))
    in_maps = []
    for c in core_ids:
        sl = slice(c * BC, (c + 1) * BC)
        sxy = np.empty((BC, NSITES * 4), np.float32)
        for t in range(NSITES):
            sxy[:, 4 * t + 0] = SX[t, sl, 1] / GSCALE
            sxy[:, 4 * t + 1] = SX[t, sl, 2] / GSCALE
            sxy[:, 4 * t + 2] = SY[t, sl, 1] / GSCALE
            sxy[:, 4 * t + 3] = SY[t, sl, 2] / GSCALE
        in_maps.append({"wgx": wgx, "wgyb": wgyb, "wgy8": wgy8, "wbf": wbf,
                        "wbx": wbx, "wby": wby, "wc7a": wc7a, "wc7b": wc7b,
                        "brow": brow, "ones0": ones0, "ident": ident,
                        "sxy": sxy})

    ntff_dir = os.environ.get("BASS_NTFF_DIR", "")
    if ntff_dir:
        os.makedirs(ntff_dir, exist_ok=True)
        _install_neff_saver(ntff_dir)
        from trn_agent_boot.trn_boot import _ntff_profile_via_ctypes
        hook = _ntff_profile_via_ctypes("/opt/axon/libaxon_pjrt.so")
        with hook(ntff_dir, None):
            res = run_bass_kernel_spmd(nc, in_maps, core_ids)
    else:
        res = run_bass_kernel_spmd(nc, in_maps, core_ids)
    last_results = res

    logits = np.concatenate(
        [res.results[c]["logits"].reshape(BC, NSITES, 6) for c in core_ids],
        axis=0)
    return _host_post(samples, logits, bl1, bl2)


# revision 26
# speedup vs baseline: 1.1680x; 1.1680x over previous
"""Trainium2 Bass kernel for the snake-ordered lattice GRU wavefunction model.

v2 strategy (data-parallel over batch, 8 cores x 128 samples):
  - 64 sequential sites; per site pre = st @ W_sel decomposed as
    base + sum_i s_i * (st @ D_i), with the bias folded into the base
    PSUM group via a ones-row k-tile (no vector bias add).
  - Contraction split by neighbor half:
      * y-half (hy, available ~15 sites early): fp8-e4m3 DoubleRow matmuls
        (ring_fp8 = 16*h, gate weights 64*D) -> 4x fewer PE cycles.
      * x-half (hx, on the critical chain): bf16 matmuls with gate weights
        pre-scaled 1024x so both halves accumulate consistently in PSUM;
        the gating scalars are divided by 1024 on the host.
  - Gating MACs read gate PSUM banks directly and are split across the
    Vector and Pool engines (2 gates each) into two partial accumulators,
    then joined quarter-wise so tanh/sigmoid can start ASAP.
  - tanh/sig/du/transpose tail runs at 128-column granularity per k-slice
    so ring k0 lands (and the next site's x GEMMs start) before the k1
    sub-chain finishes.
  - h^T is produced by PSUM-accumulated transposes T(ms)+T(u*(h~-ms));
    ring copies cast to bf16 (Vector: k0, Scalar: k1) and to fp8 (Scalar).
  - Head (h @ [Wl1|Wl2]) folded into chunk7 of the NEXT site's GEMM.
  - Softmax/sector-mask/log accumulation runs on host (O(B*64*3)).
"""
import os
import sys
import numpy as np

sys.path.insert(0, '/opt/trn_rl_repo')

B, NX, NY, I, H = 1024, 8, 8, 3, 256
N_TARGET, SZ = 48, 0
NCORES = 8
BC = B // NCORES          # 128 samples per core
NSITES = NX * NY          # 64
RING = 16                 # h ring buffer depth (max hy lookback is 15)

GSCALE = 4096.0           # x-gate weight prescale (folded out via sxy/GSCALE)
Y8_WSCALE = 64.0          # fp8 y-gate weight prescale
Y8_HSCALE = 64.0          # fp8 ring scale
USE_FP8_Y = os.environ.get("BASS_FP8Y", "0") == "1"

_cached = {}


def _snake_sites():
    sites = []
    for ny in range(NY):
        xs = range(NX) if ny % 2 == 0 else range(NX - 1, -1, -1)
        dx = -1 if ny % 2 == 0 else 1
        for nx in xs:
            sites.append((nx, ny, nx + dx))
    return sites


SITES = _snake_sites()


def _build_program():
    import concourse.tile as tile
    from concourse import bacc, mybir

    f32 = mybir.dt.float32
    bf16 = mybir.dt.bfloat16
    fp8 = mybir.dt.float8e4
    Alu = mybir.AluOpType
    Act = mybir.ActivationFunctionType
    DR = mybir.MatmulPerfMode.DoubleRow

    nc = bacc.Bacc("TRN2", target_bir_lowering=False, debug=False,
                   num_devices=NCORES)

    wgx_d = nc.dram_tensor("wgx", [256, 2048], f32, kind="ExternalInput").ap()
    wgyb_d = nc.dram_tensor("wgyb", [256, 2048], f32, kind="ExternalInput").ap()
    wgy8_d = nc.dram_tensor("wgy8", [128, 4096], f32, kind="ExternalInput").ap()
    wbf_d = nc.dram_tensor("wbf", [512, 512], f32, kind="ExternalInput").ap()
    wbx_d = nc.dram_tensor("wbx", [256, 512], f32, kind="ExternalInput").ap()
    wby_d = nc.dram_tensor("wby", [256, 512], f32, kind="ExternalInput").ap()
    wc7a_d = nc.dram_tensor("wc7a", [512, 262], f32, kind="ExternalInput").ap()
    wc7b_d = nc.dram_tensor("wc7b", [512, 262], f32, kind="ExternalInput").ap()
    brow_d = nc.dram_tensor("brow", [128, 512], f32, kind="ExternalInput").ap()
    ones0_d = nc.dram_tensor("ones0", [128, 128], f32, kind="ExternalInput").ap()
    ident_d = nc.dram_tensor("ident", [128, 128], f32, kind="ExternalInput").ap()
    sxy_d = nc.dram_tensor("sxy", [128, NSITES * 4], f32, kind="ExternalInput").ap()
    logits_d = nc.dram_tensor("logits", [128, NSITES * 6], f32,
                              kind="ExternalOutput").ap()

    with tile.TileContext(nc) as tc:
        with (
            tc.tile_pool(name="const", bufs=1) as constp,
            tc.tile_pool(name="work", bufs=4) as workp,
            tc.tile_pool(name="psc", bufs=1, space="PSUM") as pscp,
        ):
            # ---- persistent SBUF tiles ----
            wgx_sb = [constp.tile([128, 2048], bf16, tag=f"wgx{k}", name=f"wgx{k}") for k in range(2)]
            wgyb_sb = [constp.tile([128, 2048], bf16, tag=f"wgyb{k}", name=f"wgyb{k}") for k in range(2)]
            wgy8_sb = [constp.tile([128, 1024], fp8, tag=f"wgy8{c}", name=f"wgy8{c}") for c in range(4)]
            wbf_sb = [constp.tile([128, 512], bf16, tag=f"wbf{k}", name=f"wbf{k}") for k in range(4)]
            wbx_sb = [constp.tile([128, 512], bf16, tag=f"wbx{k}", name=f"wbx{k}") for k in range(2)]
            wby_sb = [constp.tile([128, 512], bf16, tag=f"wby{k}", name=f"wby{k}") for k in range(2)]
            wc7a_sb = [constp.tile([128, 262], bf16, tag=f"wa{k}", name=f"wa{k}") for k in range(4)]
            wc7b_sb = [constp.tile([128, 262], bf16, tag=f"wb{k}", name=f"wb{k}") for k in range(4)]
            brow_sb = constp.tile([128, 512], bf16, tag="brow")
            ones0_sb = constp.tile([128, 128], bf16, tag="ones0")
            zero_sb = constp.tile([128, 128], bf16, tag="zero")
            ident_sb = constp.tile([128, 128], f32, tag="ident")
            sxy_sb = constp.tile([128, NSITES * 4], f32, tag="sxy")
            ring_sb = constp.tile([128, RING * 256], bf16, tag="ring")
            ring8_sb = constp.tile([128, RING * 256], fp8, tag="ring8")
            logit_sb = constp.tile([128, NSITES * 6], f32, tag="lstage")

            # ---- persistent PSUM tiles (8 banks exactly) ----
            cb_ps = pscp.tile([128, 512], f32, tag="cb")
            g_ps = [pscp.tile([128, 512], f32, tag=f"g{c}", name=f"gps{c}")
                    for c in range(4)]
            c7_ps = [pscp.tile([128, 262], f32, tag=f"c7{i}", name=f"c7ps{i}")
                     for i in range(2)]
            tr_ps = pscp.tile([128, 256], f32, tag="tr")

            # ---- loads (f32 staging -> cast) ----
            def load_cast(dst, src_ap, k, dt):
                stg = workp.tile(list(dst.shape), f32, tag="wstage",
                                 name=f"wstage{k}", bufs=3)
                nc.sync.dma_start(stg[:], src_ap)
                nc.vector.tensor_copy(dst[:], stg[:])

            li = 0
            for k in range(2):
                rows = slice(128 * k, 128 * (k + 1))
                load_cast(wgx_sb[k], wgx_d[rows, :], li, bf16); li += 1
                load_cast(wbx_sb[k], wbx_d[rows, :], li, bf16); li += 1
                load_cast(wc7a_sb[k], wc7a_d[rows, :], li, bf16); li += 1
                load_cast(wbf_sb[k], wbf_d[rows, :], li, bf16); li += 1
            for k in range(2):
                rows = slice(128 * k, 128 * (k + 1))
                rows2 = slice(128 * (k + 2), 128 * (k + 3))
                load_cast(wgyb_sb[k], wgyb_d[rows, :], li, bf16); li += 1
                load_cast(wby_sb[k], wby_d[rows, :], li, bf16); li += 1
                load_cast(wbf_sb[k + 2], wbf_d[rows2, :], li, bf16); li += 1
                load_cast(wc7a_sb[k + 2], wc7a_d[rows2, :], li, bf16); li += 1
            for k in range(4):
                rows = slice(128 * k, 128 * (k + 1))
                load_cast(wc7b_sb[k], wc7b_d[rows, :], li, bf16); li += 1
            for c in range(4):
                load_cast(wgy8_sb[c], wgy8_d[:, 1024 * c:1024 * (c + 1)], li, fp8)
                li += 1
            load_cast(brow_sb, brow_d[:], li, bf16); li += 1
            load_cast(ones0_sb, ones0_d[:], li, bf16); li += 1
            nc.sync.dma_start(ident_sb[:], ident_d[:])
            nc.sync.dma_start(sxy_sb[:], sxy_d[:])
            nc.vector.memset(zero_sb[:], 0.0)

            def ring_k(site, k):
                base = (site % RING) * 256 + 128 * k
                return ring_sb[:, base:base + 128]

            def ring8_pair(site):
                base = (site % RING) * 256
                return ring8_sb[:, base:base + 256].rearrange(
                    "p (j b) -> p j b", j=2)

            def mm(out, lhsT, rhs, start, stop, pm=None):
                nc.tensor.matmul(out, lhsT, rhs, start=start, stop=stop,
                                 perf_mode=pm)

            # per-site state handed to the next iteration
            du_t = {}   # (du0, du1) tiles
            ms_t = {}

            for t, (nx, ny, nxn) in enumerate(SITES):
                x_act = (t % 8 != 0)
                y_act = (t >= 8)
                interior = x_act and y_act
                ta = 8 * ny - 1 - (t % 8) if y_act else -1
                w7 = wc7a_sb if (x_act or t == 0) else wc7b_sb

                def finish_prev():
                    """T(du0,t-1); T(ms1,t-1); T(du1,t-1) + ring copies.

                    Group order keeps at most one pending accumulation group
                    in the tr bank's zero region at any time.
                    """
                    pdu0, pdu1 = du_t[t - 1]
                    st0 = (t - 1 == 0)  # site0 had no ms -> start here
                    nc.tensor.matmul(tr_ps[:, 0:128], pdu0[:], ident_sb[:],
                                     is_transpose=True, start=st0, stop=True)
                    nc.vector.tensor_copy(ring_k(t - 1, 0), tr_ps[:, 0:128])
                    if not st0:
                        pms = ms_t[t - 1]
                        nc.tensor.matmul(tr_ps[:, 128:256], pms[:, 128:256],
                                         ident_sb[:], is_transpose=True,
                                         start=True, stop=False)
                    nc.tensor.matmul(tr_ps[:, 128:256], pdu1[:], ident_sb[:],
                                     is_transpose=True, start=st0, stop=True)
                    nc.vector.tensor_copy(ring_k(t - 1, 1), tr_ps[:, 128:256])
                    if USE_FP8_Y:
                        nc.scalar.activation(
                            ring8_sb[:, ((t - 1) % RING) * 256:
                                     ((t - 1) % RING) * 256 + 256],
                            tr_ps[:, 0:256], Act.Copy, scale=Y8_HSCALE)

                def fillc7(bank, n):
                    for _ in range(n):
                        mm(c7_ps[bank][:], zero_sb[:], wc7a_sb[0][:],
                           True, True)

                # ---- matmul emission ----
                if t == 0:
                    # pre = bias only; no gates, no c7
                    mm(cb_ps[:], ones0_sb[:], brow_sb[:], True, True)
                elif not y_act:
                    # x-only: contract hx (k0,k1) only
                    fillc7(t % 2, 6)
                    finish_prev()
                    xk = [ring_k(t - 1, 0), ring_k(t - 1, 1)]
                    mm(cb_ps[:], ones0_sb[:], brow_sb[:], True, False)
                    for c in (0, 1):
                        mm(g_ps[c][:], xk[0], wgx_sb[0][:, 512 * c:512 * (c + 1)],
                           True, False)
                        mm(g_ps[c][:], xk[1], wgx_sb[1][:, 512 * c:512 * (c + 1)],
                           False, True)
                    mm(cb_ps[:], xk[0], wbx_sb[0][:], False, False)
                    mm(cb_ps[:], xk[1], wbx_sb[1][:], False, True)
                    mm(c7_ps[t % 2][:], xk[0], w7[0][:], True, False)
                    mm(c7_ps[t % 2][:], xk[1], w7[1][:], False, True)
                elif not x_act:
                    # y-only: contract hy (= h(t-1)) via bf16 k0,k1 of ring
                    fillc7(t % 2, 6)
                    finish_prev()
                    yk = [ring_k(t - 1, 0), ring_k(t - 1, 1)]
                    mm(cb_ps[:], ones0_sb[:], brow_sb[:], True, False)
                    for c in (2, 3):
                        mm(g_ps[c][:], yk[0],
                           wgyb_sb[0][:, 512 * c:512 * (c + 1)], True, False)
                        mm(g_ps[c][:], yk[1],
                           wgyb_sb[1][:, 512 * c:512 * (c + 1)], False, True)
                    mm(cb_ps[:], yk[0], wby_sb[0][:], False, False)
                    mm(cb_ps[:], yk[1], wby_sb[1][:], False, True)
                    mm(c7_ps[t % 2][:], yk[0], w7[2][:], True, False)
                    mm(c7_ps[t % 2][:], yk[1], w7[3][:], False, True)
                else:
                    # interior: fillers keep the PE p-state hot through the
                    # glue window; y-window emitted in bank-free order
                    # (g0 @V-t0, g1 @V-t1, cb @V-t2, g2/g3 @S drains);
                    # then T(du,t-1); then x gate-interleaved.
                    yk = [ring_k(ta, 0), ring_k(ta, 1)]
                    xk = [ring_k(t - 1, 0), ring_k(t - 1, 1)]

                    fillc7(t % 2, 4)
                    mm(c7_ps[t % 2][:], yk[0], w7[2][:], True, False)
                    mm(c7_ps[t % 2][:], yk[1], w7[3][:], False, False)
                    fillc7((t + 1) % 2, 3)
                    for c in (0, 1):
                        mm(g_ps[c][:], yk[0],
                           wgyb_sb[0][:, 512 * c:512 * (c + 1)], True, False)
                        mm(g_ps[c][:], yk[1],
                           wgyb_sb[1][:, 512 * c:512 * (c + 1)], False, False)
                    mm(cb_ps[:], ones0_sb[:], brow_sb[:], True, False)
                    mm(cb_ps[:], yk[0], wbf_sb[2][:], False, False)
                    mm(cb_ps[:], yk[1], wbf_sb[3][:], False, False)
                    fillc7((t + 1) % 2, 3)
                    for c in (2, 3):
                        mm(g_ps[c][:], yk[0],
                           wgyb_sb[0][:, 512 * c:512 * (c + 1)], True, False)
                        mm(g_ps[c][:], yk[1],
                           wgyb_sb[1][:, 512 * c:512 * (c + 1)], False, False)
                    finish_prev()
                    # x gate-interleaved: g0/g1 close earliest for the V
                    # chain; then g2 (S drain), cb (V chain tail), g3, c7
                    mm(g_ps[0][:], xk[0], wgx_sb[0][:, 0:512], False, False)
                    mm(g_ps[1][:], xk[0], wgx_sb[0][:, 512:1024], False, False)
                    mm(g_ps[0][:], xk[1], wgx_sb[1][:, 0:512], False, True)
                    mm(g_ps[1][:], xk[1], wgx_sb[1][:, 512:1024], False, True)
                    mm(g_ps[2][:], xk[0], wgx_sb[0][:, 1024:1536], False, False)
                    mm(g_ps[2][:], xk[1], wgx_sb[1][:, 1024:1536], False, True)
                    mm(cb_ps[:], xk[0], wbf_sb[0][:], False, False)
                    mm(cb_ps[:], xk[1], wbf_sb[1][:], False, True)
                    mm(g_ps[3][:], xk[0], wgx_sb[0][:, 1536:2048], False, False)
                    mm(g_ps[3][:], xk[1], wgx_sb[1][:, 1536:2048], False, True)
                    mm(c7_ps[t % 2][:], xk[0], w7[0][:], False, False)
                    mm(c7_ps[t % 2][:], xk[1], w7[1][:], False, True)

                # ---- gating MACs (PSUM readable only by Vector + Scalar) ----
                def sc(i):
                    return sxy_sb[:, 4 * t + i:4 * t + i + 1]

                tv = workp.tile([128, 512], f32, tag="tv")
                quarters = []
                if t == 0:
                    quarters = [cb_ps[:, 0:128], cb_ps[:, 128:256],
                                cb_ps[:, 256:384], cb_ps[:, 384:512]]
                elif not interior:
                    ga, gb = (0, 1) if x_act else (2, 3)
                    nc.vector.tensor_scalar_mul(tv[:], g_ps[ga][:], sc(ga))
                    nc.vector.scalar_tensor_tensor(
                        tv[:], g_ps[gb][:], sc(gb), tv[:], Alu.mult, Alu.add)
                    nc.vector.scalar_tensor_tensor(
                        tv[:], cb_ps[:], 1.0, tv[:], Alu.mult, Alu.add)
                    quarters = [tv[:, 0:128], tv[:, 128:256],
                                tv[:, 256:384], tv[:, 384:512]]
                else:
                    gs2 = workp.tile([128, 512], f32, tag="gs2")
                    gs3 = workp.tile([128, 512], f32, tag="gs3")
                    gsum = workp.tile([128, 512], f32, tag="gsum")
                    nc.vector.tensor_scalar_mul(tv[:], g_ps[0][:], sc(0))
                    nc.vector.scalar_tensor_tensor(
                        tv[:], g_ps[1][:], sc(1), tv[:], Alu.mult, Alu.add)
                    nc.vector.scalar_tensor_tensor(
                        tv[:], cb_ps[:], 1.0, tv[:], Alu.mult, Alu.add)
                    nc.scalar.activation(gs2[:], g_ps[2][:], Act.Identity,
                                         bias=0.0, scale=sc(2))
                    nc.scalar.activation(gs3[:], g_ps[3][:], Act.Identity,
                                         bias=0.0, scale=sc(3))
                    nc.gpsimd.tensor_tensor(gsum[:, 0:256], gs2[:, 0:256],
                                            gs3[:, 0:256], Alu.add)
                    nc.gpsimd.tensor_tensor(gsum[:, 256:512], gs2[:, 256:512],
                                            gs3[:, 256:512], Alu.add)
                    a0 = workp.tile([128, 128], f32, tag="a0")
                    a1 = workp.tile([128, 128], f32, tag="a1")
                    a2 = workp.tile([128, 128], f32, tag="a2")
                    a3 = workp.tile([128, 128], f32, tag="a3")
                    nc.vector.tensor_tensor(a0[:], tv[:, 0:128], gsum[:, 0:128],
                                            Alu.add)
                    nc.vector.tensor_tensor(a2[:], tv[:, 256:384],
                                            gsum[:, 256:384], Alu.add)
                    nc.gpsimd.tensor_tensor(a1[:], tv[:, 128:256],
                                            gsum[:, 128:256], Alu.add)
                    nc.gpsimd.tensor_tensor(a3[:], tv[:, 384:512],
                                            gsum[:, 384:512], Alu.add)
                    quarters = [a0[:], a1[:], a2[:], a3[:]]

                # ---- ms + logit copies (Scalar) ----
                if t >= 1:
                    ms = workp.tile([128, 256], f32, tag="ms")
                    nc.scalar.copy(ms[:], c7_ps[t % 2][:, 0:256])
                    nc.scalar.copy(logit_sb[:, 6 * (t - 1):6 * t],
                                   c7_ps[t % 2][:, 256:262])
                    ms_t[t] = ms

                # ---- activations + du (quarter-pipelined) ----
                ht0 = workp.tile([128, 128], f32, tag="ht0")
                ht1 = workp.tile([128, 128], f32, tag="ht1")
                u0 = workp.tile([128, 128], f32, tag="u0")
                u1 = workp.tile([128, 128], f32, tag="u1")
                du0 = workp.tile([128, 128], f32, tag="du0")
                du1 = workp.tile([128, 128], f32, tag="du1")
                nc.scalar.activation(ht0[:], quarters[0], Act.Tanh)
                nc.scalar.activation(u0[:], quarters[2], Act.Sigmoid)
                if t == 0:
                    nc.vector.tensor_tensor(du0[:], ht0[:], u0[:], Alu.mult)
                else:
                    ms = ms_t[t]
                    nc.vector.tensor_tensor(du0[:], ht0[:], ms[:, 0:128],
                                            Alu.subtract)
                    nc.vector.tensor_tensor(du0[:], du0[:], u0[:], Alu.mult)
                nc.scalar.activation(ht1[:], quarters[1], Act.Tanh)
                nc.scalar.activation(u1[:], quarters[3], Act.Sigmoid)
                if t == 0:
                    nc.gpsimd.tensor_tensor(du1[:], ht1[:], u1[:], Alu.mult)
                else:
                    ms = ms_t[t]
                    nc.gpsimd.tensor_tensor(du1[:], ht1[:], ms[:, 128:256],
                                            Alu.subtract)
                    nc.gpsimd.tensor_tensor(du1[:], du1[:], u1[:], Alu.mult)
                du_t[t] = (du0, du1)

                # ---- T(ms k0) region start; k1 deferred to next finish_prev
                if t >= 1:
                    ms = ms_t[t]
                    nc.tensor.matmul(tr_ps[:, 0:128], ms[:, 0:128], ident_sb[:],
                                     is_transpose=True, start=True, stop=False)

            # ---- tail: transposes + ring + head for site 63 ----
            tl = NSITES - 1
            pdu0, pdu1 = du_t[tl]
            nc.tensor.matmul(tr_ps[:, 0:128], pdu0[:], ident_sb[:],
                             is_transpose=True, start=False, stop=True)
            nc.tensor.matmul(tr_ps[:, 128:256], ms_t[tl][:, 128:256],
                             ident_sb[:], is_transpose=True,
                             start=True, stop=False)
            nc.tensor.matmul(tr_ps[:, 128:256], pdu1[:], ident_sb[:],
                             is_transpose=True, start=False, stop=True)
            nc.vector.tensor_copy(ring_k(tl, 0), tr_ps[:, 0:128])
            nc.vector.tensor_copy(ring_k(tl, 1), tr_ps[:, 128:256])
            nc.tensor.matmul(c7_ps[0][:], ring_k(tl, 0), wc7a_sb[0][:],
                             start=True, stop=False)
            nc.tensor.matmul(c7_ps[0][:], ring_k(tl, 1), wc7a_sb[1][:],
                             start=False, stop=True)
            nc.scalar.copy(logit_sb[:, 6 * tl:6 * (tl + 1)],
                           c7_ps[0][:, 256:262])

            nc.sync.dma_start(logits_d[:], logit_sb[:])

    nc.compile()
    return nc


def _host_pre(samples, W1, W2, Wmerge, Wl1, Wl2, b1, b2):
    oh = np.zeros((B, NX, NY, I), np.float32)
    idx = np.indices(samples.shape)
    oh[idx[0], idx[1], idx[2], samples] = 1.0
    SX = np.zeros((NSITES, B, I), np.float32)
    SY = np.zeros((NSITES, B, I), np.float32)
    for t, (nx, ny, nxn) in enumerate(SITES):
        if 0 <= nxn < NX:
            SX[t] = oh[:, nxn, ny]
        if ny > 0:
            SY[t] = oh[:, nx, ny - 1]

    def DD(i, b):
        return np.concatenate([W1[i] - W1[b], W2[i] - W2[b]], axis=1)

    wg = np.concatenate([DD(1, 0), DD(2, 0), DD(4, 3), DD(5, 3)], axis=1)
    wgx = GSCALE * wg[0:256]                      # [256, 2048]
    wgyb = GSCALE * wg[256:512]                   # [256, 2048]
    # fp8 DR pack: wgy8[p, c*1024 + j*512 + o] = 64*wg[256+j*128+p, 512c+o]
    wgy8 = np.empty((128, 4096), np.float32)
    for c in range(4):
        for j in range(2):
            wgy8[:, c * 1024 + j * 512:c * 1024 + (j + 1) * 512] = \
                Y8_WSCALE * wg[256 + j * 128:256 + (j + 1) * 128,
                               512 * c:512 * (c + 1)]
    wbf = np.concatenate([W1[0] + W1[3], W2[0] + W2[3]], axis=1)
    wbx = np.concatenate([W1[0], W2[0]], axis=1)[0:256]
    wby = np.concatenate([W1[3], W2[3]], axis=1)[256:512]
    Wl = np.concatenate([Wl1, Wl2], axis=1)
    z = np.zeros((H, 6), np.float32)
    wc7a = np.concatenate([Wmerge, np.concatenate([Wl, z], 0)], axis=1)
    wc7b = np.concatenate([Wmerge, np.concatenate([z, Wl], 0)], axis=1)
    brow = np.zeros((128, 512), np.float32)
    brow[0] = np.concatenate([b1, b2])
    ones0 = np.zeros((128, 128), np.float32)
    ones0[0] = 1.0
    c = np.ascontiguousarray
    return (SX, SY, c(wgx), c(wgyb), c(wgy8), c(wbf), c(wbx), c(wby),
            c(wc7a), c(wc7b), brow, ones0)


def _host_post(samples, logits, bl1, bl2):
    """logits: [B, NSITES, 6].  Returns (0.5*log_a, log_p)."""
    log_a = np.zeros(B, np.float32)
    log_p = np.zeros(B, np.float32)
    bl_up = (N_TARGET + 2 * SZ) // 2
    bl_dn = (N_TARGET - 2 * SZ) // 2
    bl_hole = NX * NY - N_TARGET
    n_up = np.zeros(B, np.float32)
    n_dn = np.zeros(B, np.float32)
    ar = np.arange(B)
    for t, (nx, ny, nxn) in enumerate(SITES):
        l1 = logits[:, t, 0:3] + bl1
        l2 = logits[:, t, 3:6] + bl2
        e = np.exp(l1 - l1.max(axis=1, keepdims=True))
        probs = e / e.sum(axis=1, keepdims=True)
        phase = np.float32(np.pi) * (l2 / (1.0 + np.abs(l2)))
        m_up = (bl_up - n_up > 0).astype(np.float32)
        m_dn = (bl_dn - n_dn > 0).astype(np.float32)
        m_hole = (bl_hole - (t - n_up - n_dn) > 0).astype(np.float32)
        mask = np.stack([m_hole, m_dn, m_up], axis=1)
        amp = probs * mask
        amp = amp / np.maximum(amp.sum(axis=1, keepdims=True), 1e-30)
        s = samples[:, nx, ny]
        log_a += np.log(np.clip(amp[ar, s], 1e-12, None)).astype(np.float32)
        log_p += phase[ar, s].astype(np.float32)
        n_up += (s == 2)
        n_dn += (s == 1)
    return (0.5 * log_a).astype(np.float32), log_p.astype(np.float32)


last_results = None  # exposed for test.py profiling


def _install_neff_saver(dst_dir):
    """Monkeypatch bass2jax's BIR->NEFF compile to retain a NEFF copy for
    neuron-profile (the axon path normally discards it)."""
    import shutil
    from concourse import bass2jax as b2j
    if getattr(b2j, "_neff_saver_installed", False):
        return
    orig = b2j.compile_bir_kernel

    def wrapper(bir_json, tmpdir, neff_name="file.neff", **kw):
        out = orig(bir_json, tmpdir, neff_name=neff_name, **kw)
        try:
            shutil.copy(out, os.path.join(dst_dir, "kernel.neff"))
        except Exception:
            pass
        return out

    b2j.compile_bir_kernel = wrapper
    b2j._neff_saver_installed = True


def kernel(samples, W1, b1, W2, b2, Wmerge, Wl1, bl1, Wl2, bl2):
    global last_results
    from concourse.bass_utils import run_bass_kernel_spmd

    samples = np.asarray(samples).astype(np.int64)
    f = lambda x: np.asarray(x, dtype=np.float32)
    W1, b1, W2, b2 = f(W1), f(b1), f(W2), f(b2)
    Wmerge, Wl1, bl1, Wl2, bl2 = f(Wmerge), f(Wl1), f(bl1), f(Wl2), f(bl2)

    (SX, SY, wgx, wgyb, wgy8, wbf, wbx, wby, wc7a, wc7b, brow,
     ones0) = _host_pre(samples, W1, W2, Wmerge, Wl1, Wl2, b1, b2)

    if "nc" not in _cached:
        _cached["nc"] = _build_program()
    nc = _cached["nc"]

    ident = np.eye(128, dtype=np.float32)
    core_ids = list(range(NCORES))
    in_maps = []
    for c in core_ids:
        sl = slice(c * BC, (c + 1) * BC)
        sxy = np.empty((BC, NSITES * 4), np.float32)
        for t in range(NSITES):
            sxy[:, 4 * t + 0] = SX[t, sl, 1] / GSCALE
            sxy[:, 4 * t + 1] = SX[t, sl, 2] / GSCALE
            sxy[:, 4 * t + 2] = SY[t, sl, 1] / GSCALE
            sxy[:, 4 * t + 3] = SY[t, sl, 2] / GSCALE
        in_maps.append({"wgx": wgx, "wgyb": wgyb, "wgy8": wgy8, "wbf": wbf,
                        "wbx": wbx, "wby": wby, "wc7a": wc7a, "wc7b": wc7b,
                        "brow": brow, "ones0": ones0, "ident": ident,
                        "sxy": sxy})

    ntff_dir = os.environ.get("BASS_NTFF_DIR", "")
    if ntff_dir:
        os.makedirs(ntff_dir, exist_ok=True)
        _install_neff_saver(ntff_dir)
        from trn_agent_boot.trn_boot import _ntff_profile_via_ctypes
        hook = _ntff_profile_via_ctypes("/opt/axon/libaxon_pjrt.so")
        with hook(ntff_dir, None):
            res = run_bass_kernel_spmd(nc, in_maps, core_ids)
    else:
        res = run_bass_kernel_spmd(nc, in_maps, core_ids)
    last_results = res

    logits = np.concatenate(
        [res.results[c]["logits"].reshape(BC, NSITES, 6) for c in core_ids],
        axis=0)
    return _host_post(samples, logits, bl1, bl2)
